# revision 13
# baseline (speedup 1.0000x reference)
"""Trainium2 Bass kernel for nn_Attention_75849122447825 (sparse_attention).

Math: reference computes, per (b,h) head, scores x = beta * (q g)(k g)^T with a
pair mask, sparsemax over the last axis, and the scalar energy
    e = -sum_rows( <x,p> - ||p||_2 ),  output = e / beta.

Key identities (p = sparsemax(x) row-wise, sum_k p = 1):
  <x,p> = ||p||^2 + tau            (x = p + tau on the support)
  row contribution to e:  sqrt(S2) - S2 - tau,  S2 = sum relu(x - tau)^2
Masked query rows (mask[q]=0) are constant rows x = -125000; the reference's
f32 arithmetic gives them the exact constant contribution
  C = 500000 + sqrt(0.03125)   (tau_f32 = -125000.0078125, p = 2^-7 uniform,
                                support 134  ->  <x,p> = -500000, ||p|| = 2^-2.5)
so only unmasked rows need device compute; masked rows are counted on host.

tau per row via Michelot's algorithm (tau' = (sum_{x>tau} x - 1)/#{x>tau}).
The first iterate is analytic: at any tau in (-1000, min_real_x) the support is
exactly the n_u real columns, so tau1 = (rowsum + 1000*(W-n_u) - 1)/n_u comes
free from the rowsum fused into the PSUM->SBUF copy. One paid stats pass at
tau1 gives, per A tile (fused accum ops):
  cnt = #{x > tau1}                               (DVE is_gt)
  B:   m = max(x,tau1), sm = sum m   [DVE tiles]  or
       r = relu(x-tau1), sr = sum r  [ScalarE tiles]
  G:   sum (m-tau1)*m  (= S2@tau1 + tau1*(s-c*tau1))   [reads B's scratch]
       or sum r*r      (= S2@tau1)
Then batch math: s = sm - (W-c)*tau1, tau2 = (s-1)/cnt, and
S2@tau2 = S2@tau1 - 2*(tau2-tau1)*s + (tau2^2-tau1^2)*cnt (support sets are
stable at convergence) — verified to reproduce the f32 reference exactly.

Sharding: data-parallel over batch B=8, one batch per NeuronCore; host combines
the 8 partial sums plus the analytic masked-row term. Host pre-permutes each
batch's rows so unmasked rows come first and pre-transposes g. Masked key
columns get a -1000 fill through 65-row augmented projection tiles (qp row 64
= ones, kp row 64 = v). All real columns land in the leading max_b(n_u)
positions, so every elementwise/stats pass runs on a trimmed column window W
(=272 here; the graph is built for the W derived from the actual mask, cached).
The trace is emitted per 2-head group (projection -> A tiles -> tau1 -> stats)
so the engines' in-order streams pipeline across groups instead of phase
barriers.
"""

import math
import numpy as np
import ml_dtypes

import concourse.bass as bass
import concourse.tile as tile
from concourse import bacc, mybir
from concourse.bass_utils import run_bass_kernel_spmd

# problem constants (hardcoded per task rules)
B, K, D, H, Z = 8, 512, 768, 12, 64
BETA = 1.0 / math.sqrt(Z)
DC = D // 128          # 6 d-chunks
MG = (H * Z) // 128    # 6 m-groups (2 heads each)
NQC = 3                # q-row chunks of 128 processed (384 rows >= n_u always here)
NT = H * NQC           # 36 A-tiles
MASKED_ROW_E = 500000.0 + math.sqrt(0.03125)  # exact f32 reference behavior
NITERS = 1  # informational: one paid stats pass after the analytic tau1

BF16 = mybir.dt.bfloat16
F32 = mybir.dt.float32
OP = mybir.AluOpType
AF = mybir.ActivationFunctionType


def build_graph(W):
    assert W % 16 == 0 and 0 < W <= K
    nc = bacc.Bacc("TRN2", target_bir_lowering=False, debug=False,
                   enable_asserts=False, num_devices=8)

    gT_d = nc.dram_tensor("gT", [D, K], BF16, kind="ExternalInput")
    wqT_d = nc.dram_tensor("wqT", [D, H * Z], BF16, kind="ExternalInput")
    wkT_d = nc.dram_tensor("wkT", [D, H * Z], BF16, kind="ExternalInput")
    vrow_d = nc.dram_tensor("vrow", [1, K], BF16, kind="ExternalInput")
    val_d = nc.dram_tensor("val", [128, NT], F32, kind="ExternalInput")
    # params: col0 = 1000*(W-n_u)-1, col1 = 1/n_u   (replicated down partitions)
    params_d = nc.dram_tensor("params", [128, 2], F32, kind="ExternalInput")
    out_d = nc.dram_tensor("out", [1, 1], F32, kind="ExternalOutput")

    with tile.TileContext(nc) as tc:
        with (
            tc.tile_pool(name="persist", bufs=1) as pp,
            tc.tile_pool(name="scr", bufs=8) as sp,
            tc.tile_pool(name="psum", bufs=2, space="PSUM") as qpsum,
            tc.tile_pool(name="apsum", bufs=4, space="PSUM") as apsum,
            tc.tile_pool(name="tpsum", bufs=1, space="PSUM") as tpsum,
        ):
            # ---- persistent SBUF tiles ----
            gT = [pp.tile([128, K], BF16, name=f"gT{i}", tag=f"gT{i}")
                  for i in range(DC)]
            wqT = [pp.tile([128, H * Z], BF16, name=f"wqT{i}", tag=f"wqT{i}")
                   for i in range(DC)]
            wkT = [pp.tile([128, H * Z], BF16, name=f"wkT{i}", tag=f"wkT{i}")
                   for i in range(DC)]
            # 65-row augmented projections: qp row 64 = ones, kp row 64 = v
            qp = [pp.tile([65, K], BF16, name=f"qp{h}", tag=f"qp{h}")
                  for h in range(H)]
            kp = [pp.tile([65, K], BF16, name=f"kp{h}", tag=f"kp{h}")
                  for h in range(H)]
            xs = [pp.tile([128, W], BF16, name=f"x{t}", tag=f"x{t}")
                  for t in range(NT)]
            val = pp.tile([128, NT], F32, name="val", tag="val")
            params = pp.tile([128, 2], F32, name="params", tag="params")
            rowsum = pp.tile([128, NT], F32, name="rowsum", tag="rowsum")
            rs1 = pp.tile([128, NT], F32, name="rs1", tag="rs1")
            cnt = pp.tile([128, NT], F32, name="cnt", tag="cnt")
            sm = pp.tile([128, NT], F32, name="sm", tag="sm")
            sr = pp.tile([128, NT], F32, name="sr", tag="sr")
            gstat = pp.tile([128, NT], F32, name="gstat", tag="gstat")
            tau1 = pp.tile([128, NT], F32, name="tau1", tag="tau1")
            tau2 = pp.tile([128, NT], F32, name="tau2", tag="tau2")
            negtau = pp.tile([128, NT], F32, name="negtau", tag="negtau")
            sint = pp.tile([128, NT], F32, name="sint", tag="sint")
            stile = pp.tile([128, NT], F32, name="stile", tag="stile")
            sm1 = pp.tile([128, NT], F32, name="sm1", tag="sm1")
            rcp = pp.tile([128, NT], F32, name="rcp", tag="rcp")
            m2t = pp.tile([128, NT], F32, name="m2t", tag="m2t")
            cor = pp.tile([128, 12], F32, name="cor", tag="cor")
            f1t = pp.tile([128, NT], F32, name="f1t", tag="f1t")
            f2t = pp.tile([128, NT], F32, name="f2t", tag="f2t")
            g1t = pp.tile([128, NT], F32, name="g1t", tag="g1t")
            h1t = pp.tile([128, NT], F32, name="h1t", tag="h1t")
            g2t = pp.tile([128, NT], F32, name="g2t", tag="g2t")
            g3t = pp.tile([128, NT], F32, name="g3t", tag="g3t")
            s2 = pp.tile([128, NT], F32, name="s2", tag="s2")
            sq = pp.tile([128, NT], F32, name="sq", tag="sq")
            ctr = pp.tile([128, NT], F32, name="ctr", tag="ctr")
            ctr2 = pp.tile([128, NT], F32, name="ctr2", tag="ctr2")
            rowtot = pp.tile([128, 1], F32, name="rowtot", tag="rowtot")
            ones128 = pp.tile([128, 1], F32, name="ones128", tag="ones128")
            out_sb = pp.tile([1, 1], F32, name="out_sb", tag="out_sb")

            # ---- input DMAs + constants (2 HWDGE queues: sync + scalar) ----
            for i in range(DC):
                nc.sync.dma_start(gT[i][:], gT_d[i * 128:(i + 1) * 128, :])
                nc.sync.dma_start(wqT[i][:], wqT_d[i * 128:(i + 1) * 128, :])
                nc.scalar.dma_start(wkT[i][:], wkT_d[i * 128:(i + 1) * 128, :])
            for h in range(H):
                nc.gpsimd.memset(qp[h][64:65, :], 1.0)
                nc.sync.dma_start(kp[h][64:65, :], vrow_d[:])
            nc.sync.dma_start(val[:], val_d[:])
            nc.sync.dma_start(params[:], params_d[:])
            nc.vector.memset(ones128[:], 1.0)

            # ---- pipelined main loop: per 2-head group ----
            # projections -> A tiles (+rowsum copy) -> group tau1 -> stats
            for mg in range(MG):
                for w_sb, p_sb in ((wqT, qp), (wkT, kp)):
                    ps = qpsum.tile([128, K], F32, name=f"proj{mg}", tag="proj")
                    for dc in range(DC):
                        nc.tensor.matmul(
                            ps[:],
                            lhsT=w_sb[dc][:, mg * 128:(mg + 1) * 128],
                            rhs=gT[dc][:],
                            start=(dc == 0), stop=(dc == DC - 1),
                        )
                    nc.scalar.copy(p_sb[2 * mg][0:64, :], ps[0:64, :])
                    nc.scalar.copy(p_sb[2 * mg + 1][0:64, :], ps[64:128, :])

                g0 = 6 * mg
                for h in (2 * mg, 2 * mg + 1):
                    for qc in range(NQC):
                        t = h * NQC + qc
                        aps = apsum.tile([128, W], F32, name=f"a{t}", tag="a")
                        nc.tensor.matmul(
                            aps[:], lhsT=qp[h][:, qc * 128:(qc + 1) * 128],
                            rhs=kp[h][:, 0:W], start=True, stop=True)
                        if t % 3 == 0:
                            nc.vector.tensor_scalar(
                                out=xs[t][:], in0=aps[:], scalar1=0.0,
                                scalar2=None, op0=OP.add, op1=OP.add,
                                accum_out=rowsum[:, t:t + 1])
                        else:
                            nc.scalar.activation(
                                out=xs[t][:], in_=aps[:], func=AF.Identity,
                                accum_out=rowsum[:, t:t + 1])

                # group tau1 = (rowsum + 1000*(W-n_u) - 1) / n_u ; negtau
                gs = slice(g0, g0 + 6)
                nc.vector.tensor_scalar(out=rs1[:, gs], in0=rowsum[:, gs],
                                        scalar1=params[:, 0:1], scalar2=None,
                                        op0=OP.add)
                nc.vector.tensor_scalar(out=tau1[:, gs], in0=rs1[:, gs],
                                        scalar1=params[:, 1:2], scalar2=None,
                                        op0=OP.mult)
                nc.vector.tensor_scalar(out=negtau[:, gs], in0=tau1[:, gs],
                                        scalar1=-1.0, scalar2=None, op0=OP.mult)

                # stats passes at tau1
                for t in range(g0, g0 + 6):
                    nc.vector.tensor_scalar(
                        out=sp.tile([128, W], BF16, name=f"sc_{t}", tag="scr")[:],
                        in0=xs[t][:], scalar1=tau1[:, t:t + 1], scalar2=None,
                        op0=OP.is_gt, op1=OP.add, accum_out=cnt[:, t:t + 1])
                    bscr = sp.tile([128, W], BF16, name=f"sb_{t}", tag="scr")
                    if t % 3 == 0:
                        nc.vector.tensor_scalar(
                            out=bscr[:], in0=xs[t][:],
                            scalar1=tau1[:, t:t + 1], scalar2=None,
                            op0=OP.max, op1=OP.add, accum_out=sm[:, t:t + 1])
                        nc.vector.scalar_tensor_tensor(
                            out=sp.tile([128, W], BF16, name=f"sg_{t}", tag="scr")[:],
                            in0=bscr[:], scalar=tau1[:, t:t + 1], in1=bscr[:],
                            op0=OP.subtract, op1=OP.mult,
                            accum_out=gstat[:, t:t + 1])
                    else:
                        nc.scalar.activation(
                            out=bscr[:], in_=xs[t][:], func=AF.Relu,
                            bias=negtau[:, t:t + 1],
                            accum_out=sr[:, t:t + 1])
                        nc.vector.scalar_tensor_tensor(
                            out=sp.tile([128, W], BF16, name=f"sg_{t}", tag="scr")[:],
                            in0=bscr[:], scalar=0.0, in1=bscr[:],
                            op0=OP.add, op1=OP.mult,
                            accum_out=gstat[:, t:t + 1])

            # ---- batched tau2 + S2 assembly ----
            # uniformize ScalarE tiles (cols t%3 in {1,2}): sm = sr + W*tau1
            for r0 in (1, 2):
                cs = slice(r0, NT, 3)
                nc.vector.scalar_tensor_tensor(
                    out=sm[:, cs], in0=tau1[:, cs], scalar=float(W),
                    op0=OP.mult, in1=sr[:, cs], op1=OP.add)
            # sint = sm - W*tau1  (= s - cnt*tau1);  s = sint + cnt*tau1
            nc.vector.scalar_tensor_tensor(out=sint[:], in0=tau1[:],
                                           scalar=-float(W), op0=OP.mult,
                                           in1=sm[:], op1=OP.add)
            nc.vector.tensor_tensor(out=m2t[:], in0=cnt[:], in1=tau1[:],
                                    op=OP.mult)
            nc.vector.tensor_tensor(out=stile[:], in0=sint[:], in1=m2t[:],
                                    op=OP.add)
            nc.vector.tensor_scalar(out=sm1[:], in0=stile[:], scalar1=-1.0,
                                    scalar2=None, op0=OP.add)
            nc.vector.reciprocal(out=rcp[:], in_=cnt[:])
            nc.vector.tensor_tensor(out=tau2[:], in0=sm1[:], in1=rcp[:],
                                    op=OP.mult)
            # S2@tau1: DVE (max) tiles need G -= tau1*sint   (cols 0::3)
            cs = slice(0, NT, 3)
            nc.vector.tensor_tensor(out=cor[:], in0=tau1[:, cs],
                                    in1=sint[:, cs], op=OP.mult)
            nc.vector.tensor_tensor(out=gstat[:, cs], in0=gstat[:, cs],
                                    in1=cor[:], op=OP.subtract)
            # S2@tau2 = S2@tau1 - 2*(tau2-tau1)*s + (tau2^2-tau1^2)*cnt
            nc.vector.tensor_tensor(out=f1t[:], in0=tau2[:], in1=tau1[:],
                                    op=OP.subtract)
            nc.vector.tensor_tensor(out=f2t[:], in0=tau2[:], in1=tau1[:],
                                    op=OP.add)
            nc.vector.tensor_tensor(out=g1t[:], in0=f1t[:], in1=stile[:],
                                    op=OP.mult)
            nc.vector.scalar_tensor_tensor(out=h1t[:], in0=g1t[:], scalar=-2.0,
                                           op0=OP.mult, in1=gstat[:], op1=OP.add)
            nc.vector.tensor_tensor(out=g2t[:], in0=f1t[:], in1=f2t[:],
                                    op=OP.mult)
            nc.vector.tensor_tensor(out=g3t[:], in0=g2t[:], in1=cnt[:],
                                    op=OP.mult)
            nc.vector.tensor_tensor(out=s2[:], in0=h1t[:], in1=g3t[:],
                                    op=OP.add)

            # ---- epilogue: ctr = (sqrt(S2) - S2 - tau2) * valid; reduce ----
            nc.scalar.activation(out=sq[:], in_=s2[:], func=AF.Sqrt)
            nc.vector.tensor_tensor(out=ctr[:], in0=sq[:], in1=s2[:],
                                    op=OP.subtract)
            nc.vector.tensor_tensor(out=ctr2[:], in0=ctr[:], in1=tau2[:],
                                    op=OP.subtract)
            nc.vector.tensor_tensor(out=ctr[:], in0=ctr2[:], in1=val[:],
                                    op=OP.mult)
            nc.vector.tensor_reduce(out=rowtot[:], in_=ctr[:],
                                    axis=mybir.AxisListType.X, op=OP.add)
            tps = tpsum.tile([1, 1], F32, name="tot", tag="tot")
            nc.tensor.matmul(tps[:], lhsT=rowtot[:], rhs=ones128[:],
                             start=True, stop=True)
            nc.vector.tensor_copy(out_sb[:], tps[:])
            nc.sync.dma_start(out_d[:], out_sb[:])

    nc.compile()
    return nc


_NC_CACHE = {}


def _get_nc(W):
    if W not in _NC_CACHE:
        _NC_CACHE[W] = build_graph(W)
    return _NC_CACHE[W]


def window_for(mask):
    max_nu = int(mask.astype(bool).sum(1).max())
    return min(K, ((max_nu + 15) // 16) * 16)


def make_in_maps(g, wq, wk, mask):
    bf16 = ml_dtypes.bfloat16
    W = window_for(mask)
    wqT = np.ascontiguousarray(
        (wq.astype(np.float64) * BETA).transpose(2, 0, 1).reshape(D, H * Z)
    ).astype(bf16)
    wkT = np.ascontiguousarray(
        wk.transpose(2, 0, 1).reshape(D, H * Z)).astype(bf16)
    in_maps = []
    for b in range(B):
        mb = mask[b].astype(bool)
        n_u = int(mb.sum())
        assert n_u <= NQC * 128, "unmasked row count exceeds processed rows"
        perm = np.argsort(~mb, kind="stable")  # unmasked rows first
        gTp = np.ascontiguousarray(g[b].T[:, perm]).astype(bf16)
        maskp = mb[perm]
        vrow = ((maskp.astype(np.float32) - 1.0) * 1000.0)[None, :].astype(bf16)
        base = maskp[:NQC * 128].astype(np.float32).reshape(NQC, 128).T  # [128, NQC]
        val = np.ascontiguousarray(np.tile(base, (1, H)))  # cols t = h*NQC+qc
        params = np.empty((128, 2), dtype=np.float32)
        params[:, 0] = 1000.0 * (W - n_u) - 1.0
        params[:, 1] = 1.0 / n_u
        in_maps.append({"gT": gTp, "wqT": wqT, "wkT": wkT,
                        "vrow": vrow, "val": val, "params": params})
    return in_maps


def combine(partials, mask):
    n_masked_rows = H * (K - mask.sum(1).astype(np.int64))  # per batch
    total = 0.0
    for b in range(B):
        total += float(partials[b]) + MASKED_ROW_E * float(n_masked_rows[b])
    return np.asarray(total / BETA, dtype=np.float32)


def kernel(g, wq, wk, mask):
    mask = np.asarray(mask)
    nc = _get_nc(window_for(mask))
    in_maps = make_in_maps(np.asarray(g, dtype=np.float32),
                           np.asarray(wq, dtype=np.float32),
                           np.asarray(wk, dtype=np.float32),
                           mask)
    res = run_bass_kernel_spmd(nc, in_maps, core_ids=list(range(8)))
    partials = [np.asarray(res.results[b]["out"], dtype=np.float64).reshape(-1)[0]
                for b in range(B)]
    return combine(partials, mask)


# revision 14
# speedup vs baseline: 1.0026x; 1.0026x over previous
"""Trainium2 Bass kernel for nn_Attention_75849122447825 (sparse_attention).

Math: reference computes, per (b,h) head, scores x = beta * (q g)(k g)^T with a
pair mask, sparsemax over the last axis, and the scalar energy
    e = -sum_rows( <x,p> - ||p||_2 ),  output = e / beta.

Key identities (p = sparsemax(x) row-wise, sum_k p = 1):
  <x,p> = ||p||^2 + tau            (x = p + tau on the support)
  row contribution to e:  sqrt(S2) - S2 - tau,  S2 = sum relu(x - tau)^2
Masked query rows (mask[q]=0) are constant rows x = -125000; the reference's
f32 arithmetic gives them the exact constant contribution
  C = 500000 + sqrt(0.03125)   (tau_f32 = -125000.0078125, p = 2^-7 uniform,
                                support 134  ->  <x,p> = -500000, ||p|| = 2^-2.5)
so only unmasked rows need device compute; masked rows are counted on host.

tau per row via Michelot's algorithm (tau' = (sum_{x>tau} x - 1)/#{x>tau}).
The first iterate is analytic: at any tau in (-1000, min_real_x) the support is
exactly the n_u real columns, so tau1 = (rowsum + 1000*(W-n_u) - 1)/n_u comes
free from the rowsum fused into the PSUM->SBUF copy. One paid stats pass at
tau1 gives, per A tile (fused accum ops):
  cnt = #{x > tau1}                               (DVE is_gt)
  B:   m = max(x,tau1), sm = sum m   [DVE tiles]  or
       r = relu(x-tau1), sr = sum r  [ScalarE tiles]
  G:   sum (m-tau1)*m  (= S2@tau1 + tau1*(s-c*tau1))   [reads B's scratch]
       or sum r*r      (= S2@tau1)
Then batch math: s = sm - (W-c)*tau1, tau2 = (s-1)/cnt, and
S2@tau2 = S2@tau1 - 2*(tau2-tau1)*s + (tau2^2-tau1^2)*cnt (support sets are
stable at convergence) — verified to reproduce the f32 reference exactly.

Sharding: data-parallel over batch B=8, one batch per NeuronCore; host combines
the 8 partial sums plus the analytic masked-row term. Host pre-permutes each
batch's rows so unmasked rows come first and pre-transposes g. Masked key
columns get a -1000 fill through 65-row augmented projection tiles (qp row 64
= ones, kp row 64 = v). All real columns land in the leading max_b(n_u)
positions, so every elementwise/stats pass runs on a trimmed column window W
(=272 here; the graph is built for the W derived from the actual mask, cached).
The trace is emitted per 2-head group (projection -> A tiles -> tau1 -> stats)
so the engines' in-order streams pipeline across groups instead of phase
barriers.
"""

import math
import numpy as np
import ml_dtypes

import concourse.bass as bass
import concourse.tile as tile
from concourse import bacc, mybir
from concourse.bass_utils import run_bass_kernel_spmd

# problem constants (hardcoded per task rules)
B, K, D, H, Z = 8, 512, 768, 12, 64
BETA = 1.0 / math.sqrt(Z)
DC = D // 128          # 6 d-chunks
MG = (H * Z) // 128    # 6 m-groups (2 heads each)
NQC = 3                # q-row chunks of 128 processed (384 rows >= n_u always here)
NT = H * NQC           # 36 A-tiles
MASKED_ROW_E = 500000.0 + math.sqrt(0.03125)  # exact f32 reference behavior
NITERS = 1  # informational: one paid stats pass after the analytic tau1

BF16 = mybir.dt.bfloat16
F32 = mybir.dt.float32
OP = mybir.AluOpType
AF = mybir.ActivationFunctionType


def build_graph(W):
    assert W % 16 == 0 and 0 < W <= K
    nc = bacc.Bacc("TRN2", target_bir_lowering=False, debug=False,
                   enable_asserts=False, num_devices=8)

    gT_d = nc.dram_tensor("gT", [D, K], BF16, kind="ExternalInput")
    wqT_d = nc.dram_tensor("wqT", [D, H * Z], BF16, kind="ExternalInput")
    wkT_d = nc.dram_tensor("wkT", [D, H * Z], BF16, kind="ExternalInput")
    vrow_d = nc.dram_tensor("vrow", [1, K], BF16, kind="ExternalInput")
    val_d = nc.dram_tensor("val", [128, NT], F32, kind="ExternalInput")
    # params: col0 = 1000*(W-n_u)-1, col1 = 1/n_u   (replicated down partitions)
    params_d = nc.dram_tensor("params", [128, 2], F32, kind="ExternalInput")
    out_d = nc.dram_tensor("out", [1, 1], F32, kind="ExternalOutput")

    with tile.TileContext(nc) as tc:
        with (
            tc.tile_pool(name="persist", bufs=1) as pp,
            tc.tile_pool(name="scr", bufs=8) as sp,
            tc.tile_pool(name="psum", bufs=2, space="PSUM") as qpsum,
            tc.tile_pool(name="apsum", bufs=4, space="PSUM") as apsum,
            tc.tile_pool(name="tpsum", bufs=1, space="PSUM") as tpsum,
        ):
            # ---- persistent SBUF tiles ----
            gT = [pp.tile([128, K], BF16, name=f"gT{i}", tag=f"gT{i}")
                  for i in range(DC)]
            wqT = [pp.tile([128, H * Z], BF16, name=f"wqT{i}", tag=f"wqT{i}")
                   for i in range(DC)]
            wkT = [pp.tile([128, H * Z], BF16, name=f"wkT{i}", tag=f"wkT{i}")
                   for i in range(DC)]
            # 65-row augmented projections: qp row 64 = ones, kp row 64 = v
            QCOLS = NQC * 128
            qp = [pp.tile([65, QCOLS], BF16, name=f"qp{h}", tag=f"qp{h}")
                  for h in range(H)]
            kp = [pp.tile([65, W], BF16, name=f"kp{h}", tag=f"kp{h}")
                  for h in range(H)]
            xs = [pp.tile([128, W], BF16, name=f"x{t}", tag=f"x{t}")
                  for t in range(NT)]
            val = pp.tile([128, NT], F32, name="val", tag="val")
            params = pp.tile([128, 2], F32, name="params", tag="params")
            rowsum = pp.tile([128, NT], F32, name="rowsum", tag="rowsum")
            rs1 = pp.tile([128, NT], F32, name="rs1", tag="rs1")
            cnt = pp.tile([128, NT], F32, name="cnt", tag="cnt")
            sm = pp.tile([128, NT], F32, name="sm", tag="sm")
            sr = pp.tile([128, NT], F32, name="sr", tag="sr")
            gstat = pp.tile([128, NT], F32, name="gstat", tag="gstat")
            tau1 = pp.tile([128, NT], F32, name="tau1", tag="tau1")
            tau2 = pp.tile([128, NT], F32, name="tau2", tag="tau2")
            negtau = pp.tile([128, NT], F32, name="negtau", tag="negtau")
            sint = pp.tile([128, NT], F32, name="sint", tag="sint")
            stile = pp.tile([128, NT], F32, name="stile", tag="stile")
            sm1 = pp.tile([128, NT], F32, name="sm1", tag="sm1")
            rcp = pp.tile([128, NT], F32, name="rcp", tag="rcp")
            m2t = pp.tile([128, NT], F32, name="m2t", tag="m2t")
            cor = pp.tile([128, 12], F32, name="cor", tag="cor")
            f1t = pp.tile([128, NT], F32, name="f1t", tag="f1t")
            f2t = pp.tile([128, NT], F32, name="f2t", tag="f2t")
            g1t = pp.tile([128, NT], F32, name="g1t", tag="g1t")
            h1t = pp.tile([128, NT], F32, name="h1t", tag="h1t")
            g2t = pp.tile([128, NT], F32, name="g2t", tag="g2t")
            g3t = pp.tile([128, NT], F32, name="g3t", tag="g3t")
            s2 = pp.tile([128, NT], F32, name="s2", tag="s2")
            sq = pp.tile([128, NT], F32, name="sq", tag="sq")
            ctr = pp.tile([128, NT], F32, name="ctr", tag="ctr")
            ctr2 = pp.tile([128, NT], F32, name="ctr2", tag="ctr2")
            rowtot = pp.tile([128, 1], F32, name="rowtot", tag="rowtot")
            ones128 = pp.tile([128, 1], F32, name="ones128", tag="ones128")
            out_sb = pp.tile([1, 1], F32, name="out_sb", tag="out_sb")

            # ---- input DMAs + constants ----
            for i in range(DC):
                nc.sync.dma_start(gT[i][:], gT_d[i * 128:(i + 1) * 128, :])
                nc.sync.dma_start(wqT[i][:], wqT_d[i * 128:(i + 1) * 128, :])
                nc.sync.dma_start(wkT[i][:], wkT_d[i * 128:(i + 1) * 128, :])
            nc.sync.dma_start(val[:], val_d[:])
            nc.sync.dma_start(params[:], params_d[:])
            nc.vector.memset(ones128[:], 1.0)
            for h in range(H):
                nc.gpsimd.memset(qp[h][64:65, 0:QCOLS], 1.0)
                nc.sync.dma_start(kp[h][64:65, 0:W], vrow_d[0:1, 0:W])

            # ---- pipelined main loop: per 2-head group ----
            # proj(mg+1) is emitted before stats(mg) so ACT's proj copies are
            # not stuck behind the previous group's relu passes
            def emit_proj(mg):
                for w_sb, p_sb, ncols in ((wqT, qp, QCOLS), (wkT, kp, W)):
                    ps = qpsum.tile([128, ncols], F32,
                                    name=f"proj{mg}_{ncols}", tag="proj")
                    for dc in range(DC):
                        nc.tensor.matmul(
                            ps[:],
                            lhsT=w_sb[dc][:, mg * 128:(mg + 1) * 128],
                            rhs=gT[dc][:, 0:ncols],
                            start=(dc == 0), stop=(dc == DC - 1),
                        )
                    nc.scalar.copy(p_sb[2 * mg][0:64, :], ps[0:64, :])
                    nc.scalar.copy(p_sb[2 * mg + 1][0:64, :], ps[64:128, :])

            emit_proj(0)
            for mg in range(MG):
                g0 = 6 * mg
                for h in (2 * mg, 2 * mg + 1):
                    for qc in range(NQC):
                        t = h * NQC + qc
                        aps = apsum.tile([128, W], F32, name=f"a{t}", tag="a")
                        nc.tensor.matmul(
                            aps[:], lhsT=qp[h][:, qc * 128:(qc + 1) * 128],
                            rhs=kp[h][:], start=True, stop=True)
                        if t % 3 == 0:
                            nc.vector.tensor_scalar(
                                out=xs[t][:], in0=aps[:], scalar1=0.0,
                                scalar2=None, op0=OP.add, op1=OP.add,
                                accum_out=rowsum[:, t:t + 1])
                        else:
                            nc.scalar.activation(
                                out=xs[t][:], in_=aps[:], func=AF.Identity,
                                accum_out=rowsum[:, t:t + 1])

                if mg + 1 < MG:
                    emit_proj(mg + 1)

                # group tau1 = (rowsum + 1000*(W-n_u) - 1) / n_u ; negtau
                gs = slice(g0, g0 + 6)
                nc.vector.tensor_scalar(out=rs1[:, gs], in0=rowsum[:, gs],
                                        scalar1=params[:, 0:1], scalar2=None,
                                        op0=OP.add)
                nc.vector.tensor_scalar(out=tau1[:, gs], in0=rs1[:, gs],
                                        scalar1=params[:, 1:2], scalar2=None,
                                        op0=OP.mult)
                nc.vector.tensor_scalar(out=negtau[:, gs], in0=tau1[:, gs],
                                        scalar1=-1.0, scalar2=None, op0=OP.mult)

                # stats passes at tau1
                for t in range(g0, g0 + 6):
                    nc.vector.tensor_scalar(
                        out=sp.tile([128, W], BF16, name=f"sc_{t}", tag="scr")[:],
                        in0=xs[t][:], scalar1=tau1[:, t:t + 1], scalar2=None,
                        op0=OP.is_gt, op1=OP.add, accum_out=cnt[:, t:t + 1])
                    bscr = sp.tile([128, W], BF16, name=f"sb_{t}", tag="scr")
                    if t % 3 == 0:
                        nc.vector.tensor_scalar(
                            out=bscr[:], in0=xs[t][:],
                            scalar1=tau1[:, t:t + 1], scalar2=None,
                            op0=OP.max, op1=OP.add, accum_out=sm[:, t:t + 1])
                        nc.vector.scalar_tensor_tensor(
                            out=sp.tile([128, W], BF16, name=f"sg_{t}", tag="scr")[:],
                            in0=bscr[:], scalar=tau1[:, t:t + 1], in1=bscr[:],
                            op0=OP.subtract, op1=OP.mult,
                            accum_out=gstat[:, t:t + 1])
                    else:
                        nc.scalar.activation(
                            out=bscr[:], in_=xs[t][:], func=AF.Relu,
                            bias=negtau[:, t:t + 1],
                            accum_out=sr[:, t:t + 1])
                        nc.vector.scalar_tensor_tensor(
                            out=sp.tile([128, W], BF16, name=f"sg_{t}", tag="scr")[:],
                            in0=bscr[:], scalar=0.0, in1=bscr[:],
                            op0=OP.add, op1=OP.mult,
                            accum_out=gstat[:, t:t + 1])

            # ---- batched tau2 + S2 assembly ----
            # uniformize ScalarE tiles (cols t%3 in {1,2}): sm = sr + W*tau1
            for r0 in (1, 2):
                cs = slice(r0, NT, 3)
                nc.vector.scalar_tensor_tensor(
                    out=sm[:, cs], in0=tau1[:, cs], scalar=float(W),
                    op0=OP.mult, in1=sr[:, cs], op1=OP.add)
            # sint = sm - W*tau1  (= s - cnt*tau1);  s = sint + cnt*tau1
            nc.vector.scalar_tensor_tensor(out=sint[:], in0=tau1[:],
                                           scalar=-float(W), op0=OP.mult,
                                           in1=sm[:], op1=OP.add)
            nc.vector.tensor_tensor(out=m2t[:], in0=cnt[:], in1=tau1[:],
                                    op=OP.mult)
            nc.vector.tensor_tensor(out=stile[:], in0=sint[:], in1=m2t[:],
                                    op=OP.add)
            nc.vector.tensor_scalar(out=sm1[:], in0=stile[:], scalar1=-1.0,
                                    scalar2=None, op0=OP.add)
            nc.vector.reciprocal(out=rcp[:], in_=cnt[:])
            nc.vector.tensor_tensor(out=tau2[:], in0=sm1[:], in1=rcp[:],
                                    op=OP.mult)
            # S2@tau1: DVE (max) tiles need G -= tau1*sint   (cols 0::3)
            cs = slice(0, NT, 3)
            nc.vector.tensor_tensor(out=cor[:], in0=tau1[:, cs],
                                    in1=sint[:, cs], op=OP.mult)
            nc.vector.tensor_tensor(out=gstat[:, cs], in0=gstat[:, cs],
                                    in1=cor[:], op=OP.subtract)
            # S2@tau2 = S2@tau1 - 2*(tau2-tau1)*s + (tau2^2-tau1^2)*cnt
            nc.vector.tensor_tensor(out=f1t[:], in0=tau2[:], in1=tau1[:],
                                    op=OP.subtract)
            nc.vector.tensor_tensor(out=f2t[:], in0=tau2[:], in1=tau1[:],
                                    op=OP.add)
            nc.vector.tensor_tensor(out=g1t[:], in0=f1t[:], in1=stile[:],
                                    op=OP.mult)
            nc.vector.scalar_tensor_tensor(out=h1t[:], in0=g1t[:], scalar=-2.0,
                                           op0=OP.mult, in1=gstat[:], op1=OP.add)
            nc.vector.tensor_tensor(out=g2t[:], in0=f1t[:], in1=f2t[:],
                                    op=OP.mult)
            nc.vector.tensor_tensor(out=g3t[:], in0=g2t[:], in1=cnt[:],
                                    op=OP.mult)
            nc.vector.tensor_tensor(out=s2[:], in0=h1t[:], in1=g3t[:],
                                    op=OP.add)

            # ---- epilogue: ctr = (sqrt(S2) - S2 - tau2) * valid; reduce ----
            nc.scalar.activation(out=sq[:], in_=s2[:], func=AF.Sqrt)
            nc.vector.tensor_tensor(out=ctr[:], in0=sq[:], in1=s2[:],
                                    op=OP.subtract)
            nc.vector.tensor_tensor(out=ctr2[:], in0=ctr[:], in1=tau2[:],
                                    op=OP.subtract)
            nc.vector.tensor_tensor(out=ctr[:], in0=ctr2[:], in1=val[:],
                                    op=OP.mult)
            nc.vector.tensor_reduce(out=rowtot[:], in_=ctr[:],
                                    axis=mybir.AxisListType.X, op=OP.add)
            tps = tpsum.tile([1, 1], F32, name="tot", tag="tot")
            nc.tensor.matmul(tps[:], lhsT=rowtot[:], rhs=ones128[:],
                             start=True, stop=True)
            nc.vector.tensor_copy(out_sb[:], tps[:])
            nc.sync.dma_start(out_d[:], out_sb[:])

    nc.compile()
    return nc


_NC_CACHE = {}


def _get_nc(W):
    if W not in _NC_CACHE:
        _NC_CACHE[W] = build_graph(W)
    return _NC_CACHE[W]


def window_for(mask):
    max_nu = int(mask.astype(bool).sum(1).max())
    return min(K, ((max_nu + 15) // 16) * 16)


def make_in_maps(g, wq, wk, mask):
    bf16 = ml_dtypes.bfloat16
    W = window_for(mask)
    wqT = np.ascontiguousarray(
        (wq.astype(np.float64) * BETA).transpose(2, 0, 1).reshape(D, H * Z)
    ).astype(bf16)
    wkT = np.ascontiguousarray(
        wk.transpose(2, 0, 1).reshape(D, H * Z)).astype(bf16)
    in_maps = []
    for b in range(B):
        mb = mask[b].astype(bool)
        n_u = int(mb.sum())
        assert n_u <= NQC * 128, "unmasked row count exceeds processed rows"
        perm = np.argsort(~mb, kind="stable")  # unmasked rows first
        gTp = np.ascontiguousarray(g[b].T[:, perm]).astype(bf16)
        maskp = mb[perm]
        vrow = ((maskp.astype(np.float32) - 1.0) * 1000.0)[None, :].astype(bf16)
        base = maskp[:NQC * 128].astype(np.float32).reshape(NQC, 128).T  # [128, NQC]
        val = np.ascontiguousarray(np.tile(base, (1, H)))  # cols t = h*NQC+qc
        params = np.empty((128, 2), dtype=np.float32)
        params[:, 0] = 1000.0 * (W - n_u) - 1.0
        params[:, 1] = 1.0 / n_u
        in_maps.append({"gT": gTp, "wqT": wqT, "wkT": wkT,
                        "vrow": vrow, "val": val, "params": params})
    return in_maps


def combine(partials, mask):
    n_masked_rows = H * (K - mask.sum(1).astype(np.int64))  # per batch
    total = 0.0
    for b in range(B):
        total += float(partials[b]) + MASKED_ROW_E * float(n_masked_rows[b])
    return np.asarray(total / BETA, dtype=np.float32)


def kernel(g, wq, wk, mask):
    mask = np.asarray(mask)
    nc = _get_nc(window_for(mask))
    in_maps = make_in_maps(np.asarray(g, dtype=np.float32),
                           np.asarray(wq, dtype=np.float32),
                           np.asarray(wk, dtype=np.float32),
                           mask)
    res = run_bass_kernel_spmd(nc, in_maps, core_ids=list(range(8)))
    partials = [np.asarray(res.results[b]["out"], dtype=np.float64).reshape(-1)[0]
                for b in range(B)]
    return combine(partials, mask)


# revision 15
# speedup vs baseline: 1.0219x; 1.0192x over previous
"""Trainium2 Bass kernel for nn_Attention_75849122447825 (sparse_attention).

Math: reference computes, per (b,h) head, scores x = beta * (q g)(k g)^T with a
pair mask, sparsemax over the last axis, and the scalar energy
    e = -sum_rows( <x,p> - ||p||_2 ),  output = e / beta.

Key identities (p = sparsemax(x) row-wise, sum_k p = 1):
  <x,p> = ||p||^2 + tau            (x = p + tau on the support)
  row contribution to e:  sqrt(S2) - S2 - tau,  S2 = sum relu(x - tau)^2
Masked query rows (mask[q]=0) are constant rows x = -125000; the reference's
f32 arithmetic gives them the exact constant contribution
  C = 500000 + sqrt(0.03125)   (tau_f32 = -125000.0078125, p = 2^-7 uniform,
                                support 134  ->  <x,p> = -500000, ||p|| = 2^-2.5)
so only unmasked rows need device compute; masked rows are counted on host.

tau per row via Michelot's algorithm (tau' = (sum_{x>tau} x - 1)/#{x>tau}).
The first iterate is analytic: at any tau in (-1000, min_real_x) the support is
exactly the n_u real columns, so tau1 = (rowsum + 1000*(W-n_u) - 1)/n_u comes
free from the rowsum fused into the PSUM->SBUF copy. One paid stats pass at
tau1 gives, per A tile (fused accum ops):
  cnt = #{x > tau1}                               (DVE is_gt)
  B:   m = max(x,tau1), sm = sum m   [DVE tiles]  or
       r = relu(x-tau1), sr = sum r  [ScalarE tiles]
  G:   sum (m-tau1)*m  (= S2@tau1 + tau1*(s-c*tau1))   [reads B's scratch]
       or sum r*r      (= S2@tau1)
Then batch math: s = sm - (W-c)*tau1, tau2 = (s-1)/cnt, and
S2@tau2 = S2@tau1 - 2*(tau2-tau1)*s + (tau2^2-tau1^2)*cnt (support sets are
stable at convergence) — verified to reproduce the f32 reference exactly.

Sharding: data-parallel over batch B=8, one batch per NeuronCore; host combines
the 8 partial sums plus the analytic masked-row term. Host pre-permutes each
batch's rows so unmasked rows come first and pre-transposes g. Masked key
columns get a -1000 fill through 65-row augmented projection tiles (qp row 64
= ones, kp row 64 = v). All real columns land in the leading max_b(n_u)
positions, so every elementwise/stats pass runs on a trimmed column window W
(=272 here; the graph is built for the W derived from the actual mask, cached).
The trace is emitted per 2-head group (projection -> A tiles -> tau1 -> stats)
so the engines' in-order streams pipeline across groups instead of phase
barriers.
"""

import math
import numpy as np
import ml_dtypes

import concourse.bass as bass
import concourse.tile as tile
from concourse import bacc, mybir
from concourse.bass_utils import run_bass_kernel_spmd

# problem constants (hardcoded per task rules)
B, K, D, H, Z = 8, 512, 768, 12, 64
BETA = 1.0 / math.sqrt(Z)
DC = D // 128          # 6 d-chunks
MG = (H * Z) // 128    # 6 m-groups (2 heads each)
NQC = 3                # q-row chunks of 128 processed (384 rows >= n_u always here)
NT = H * NQC           # 36 A-tiles
MASKED_ROW_E = 500000.0 + math.sqrt(0.03125)  # exact f32 reference behavior
NITERS = 1  # informational: one paid stats pass after the analytic tau1

BF16 = mybir.dt.bfloat16
F32 = mybir.dt.float32
OP = mybir.AluOpType
AF = mybir.ActivationFunctionType


def build_graph(W):
    assert W % 16 == 0 and 0 < W <= K
    nc = bacc.Bacc("TRN2", target_bir_lowering=False, debug=False,
                   enable_asserts=False, num_devices=8)

    gT_d = nc.dram_tensor("gT", [D, K], BF16, kind="ExternalInput")
    wqT_d = nc.dram_tensor("wqT", [D, H * Z], BF16, kind="ExternalInput")
    wkT_d = nc.dram_tensor("wkT", [D, H * Z], BF16, kind="ExternalInput")
    vrow_d = nc.dram_tensor("vrow", [1, K], BF16, kind="ExternalInput")
    val_d = nc.dram_tensor("val", [128, NT], F32, kind="ExternalInput")
    # params: col0 = 1000*(W-n_u)-1, col1 = 1/n_u   (replicated down partitions)
    params_d = nc.dram_tensor("params", [128, 2], F32, kind="ExternalInput")
    out_d = nc.dram_tensor("out", [1, 1], F32, kind="ExternalOutput")

    with tile.TileContext(nc) as tc:
        with (
            tc.tile_pool(name="persist", bufs=1) as pp,
            tc.tile_pool(name="scr", bufs=8) as sp,
            tc.tile_pool(name="psum", bufs=2, space="PSUM") as qpsum,
            tc.tile_pool(name="apsum", bufs=5, space="PSUM") as apsum,
            tc.tile_pool(name="tpsum", bufs=1, space="PSUM") as tpsum,
        ):
            # ---- persistent SBUF tiles ----
            gT = [pp.tile([128, K], BF16, name=f"gT{i}", tag=f"gT{i}")
                  for i in range(DC)]
            wqT = [pp.tile([128, H * Z], BF16, name=f"wqT{i}", tag=f"wqT{i}")
                   for i in range(DC)]
            wkT = [pp.tile([128, H * Z], BF16, name=f"wkT{i}", tag=f"wkT{i}")
                   for i in range(DC)]
            # 65-row augmented projections: qp row 64 = ones, kp row 64 = v
            QCOLS = NQC * 128
            qp = [pp.tile([65, QCOLS], BF16, name=f"qp{h}", tag=f"qp{h}")
                  for h in range(H)]
            kp = [pp.tile([65, W], BF16, name=f"kp{h}", tag=f"kp{h}")
                  for h in range(H)]
            xs = [pp.tile([128, W], BF16, name=f"x{t}", tag=f"x{t}")
                  for t in range(NT)]
            val = pp.tile([128, NT], F32, name="val", tag="val")
            params = pp.tile([128, 2], F32, name="params", tag="params")
            rowsum = pp.tile([128, NT], F32, name="rowsum", tag="rowsum")
            rs1 = pp.tile([128, NT], F32, name="rs1", tag="rs1")
            cnt = pp.tile([128, NT], F32, name="cnt", tag="cnt")
            sm = pp.tile([128, NT], F32, name="sm", tag="sm")
            sr = pp.tile([128, NT], F32, name="sr", tag="sr")
            gstat = pp.tile([128, NT], F32, name="gstat", tag="gstat")
            tau1 = pp.tile([128, NT], F32, name="tau1", tag="tau1")
            tau2 = pp.tile([128, NT], F32, name="tau2", tag="tau2")
            negtau = pp.tile([128, NT], F32, name="negtau", tag="negtau")
            sint = pp.tile([128, NT], F32, name="sint", tag="sint")
            stile = pp.tile([128, NT], F32, name="stile", tag="stile")
            sm1 = pp.tile([128, NT], F32, name="sm1", tag="sm1")
            rcp = pp.tile([128, NT], F32, name="rcp", tag="rcp")
            m2t = pp.tile([128, NT], F32, name="m2t", tag="m2t")
            cor = pp.tile([128, 12], F32, name="cor", tag="cor")
            f1t = pp.tile([128, NT], F32, name="f1t", tag="f1t")
            f2t = pp.tile([128, NT], F32, name="f2t", tag="f2t")
            g1t = pp.tile([128, NT], F32, name="g1t", tag="g1t")
            h1t = pp.tile([128, NT], F32, name="h1t", tag="h1t")
            g2t = pp.tile([128, NT], F32, name="g2t", tag="g2t")
            g3t = pp.tile([128, NT], F32, name="g3t", tag="g3t")
            s2 = pp.tile([128, NT], F32, name="s2", tag="s2")
            sq = pp.tile([128, NT], F32, name="sq", tag="sq")
            ctr = pp.tile([128, NT], F32, name="ctr", tag="ctr")
            ctr2 = pp.tile([128, NT], F32, name="ctr2", tag="ctr2")
            rowtot = pp.tile([128, 1], F32, name="rowtot", tag="rowtot")
            ones128 = pp.tile([128, 1], F32, name="ones128", tag="ones128")
            out_sb = pp.tile([1, 1], F32, name="out_sb", tag="out_sb")

            # ---- input DMAs + constants ----
            for i in range(DC):
                nc.sync.dma_start(gT[i][:], gT_d[i * 128:(i + 1) * 128, :])
                nc.sync.dma_start(wqT[i][:], wqT_d[i * 128:(i + 1) * 128, :])
                nc.sync.dma_start(wkT[i][:], wkT_d[i * 128:(i + 1) * 128, :])
            nc.sync.dma_start(val[:], val_d[:])
            nc.sync.dma_start(params[:], params_d[:])
            nc.vector.memset(ones128[:], 1.0)
            for h in range(H):
                nc.gpsimd.memset(qp[h][64:65, 0:QCOLS], 1.0)
                nc.sync.dma_start(kp[h][64:65, 0:W], vrow_d[0:1, 0:W])

            # ---- pipelined main loop: per 2-head group ----
            # proj(mg+1) is emitted before stats(mg) so ACT's proj copies are
            # not stuck behind the previous group's relu passes
            def emit_proj(mg):
                for w_sb, p_sb, ncols in ((wqT, qp, QCOLS), (wkT, kp, W)):
                    ps = qpsum.tile([128, ncols], F32,
                                    name=f"proj{mg}_{ncols}", tag="proj")
                    for dc in range(DC):
                        nc.tensor.matmul(
                            ps[:],
                            lhsT=w_sb[dc][:, mg * 128:(mg + 1) * 128],
                            rhs=gT[dc][:, 0:ncols],
                            start=(dc == 0), stop=(dc == DC - 1),
                        )
                    nc.scalar.copy(p_sb[2 * mg][0:64, :], ps[0:64, :])
                    nc.scalar.copy(p_sb[2 * mg + 1][0:64, :], ps[64:128, :])

            emit_proj(0)
            for mg in range(MG):
                g0 = 6 * mg
                for h in (2 * mg, 2 * mg + 1):
                    for qc in range(NQC):
                        t = h * NQC + qc
                        aps = apsum.tile([128, W], F32, name=f"a{t}", tag="a")
                        nc.tensor.matmul(
                            aps[:], lhsT=qp[h][:, qc * 128:(qc + 1) * 128],
                            rhs=kp[h][:], start=True, stop=True)
                        if t % 3 == 0:
                            nc.vector.tensor_scalar(
                                out=xs[t][:], in0=aps[:], scalar1=0.0,
                                scalar2=None, op0=OP.add, op1=OP.add,
                                accum_out=rowsum[:, t:t + 1])
                        else:
                            nc.scalar.activation(
                                out=xs[t][:], in_=aps[:], func=AF.Identity,
                                accum_out=rowsum[:, t:t + 1])

                if mg + 1 < MG:
                    emit_proj(mg + 1)

                # group tau1 = (rowsum + 1000*(W-n_u) - 1) / n_u ; negtau
                gs = slice(g0, g0 + 6)
                nc.vector.tensor_scalar(out=rs1[:, gs], in0=rowsum[:, gs],
                                        scalar1=params[:, 0:1], scalar2=None,
                                        op0=OP.add)
                nc.vector.tensor_scalar(out=tau1[:, gs], in0=rs1[:, gs],
                                        scalar1=params[:, 1:2], scalar2=None,
                                        op0=OP.mult)
                nc.vector.tensor_scalar(out=negtau[:, gs], in0=tau1[:, gs],
                                        scalar1=-1.0, scalar2=None, op0=OP.mult)

                # stats passes at tau1
                for t in range(g0, g0 + 6):
                    nc.vector.tensor_scalar(
                        out=sp.tile([128, W], BF16, name=f"sc_{t}", tag="scr")[:],
                        in0=xs[t][:], scalar1=tau1[:, t:t + 1], scalar2=None,
                        op0=OP.is_gt, op1=OP.add, accum_out=cnt[:, t:t + 1])
                    bscr = sp.tile([128, W], BF16, name=f"sb_{t}", tag="scr")
                    if t % 3 == 0:
                        nc.vector.tensor_scalar(
                            out=bscr[:], in0=xs[t][:],
                            scalar1=tau1[:, t:t + 1], scalar2=None,
                            op0=OP.max, op1=OP.add, accum_out=sm[:, t:t + 1])
                        nc.vector.scalar_tensor_tensor(
                            out=sp.tile([128, W], BF16, name=f"sg_{t}", tag="scr")[:],
                            in0=bscr[:], scalar=tau1[:, t:t + 1], in1=bscr[:],
                            op0=OP.subtract, op1=OP.mult,
                            accum_out=gstat[:, t:t + 1])
                    else:
                        nc.scalar.activation(
                            out=bscr[:], in_=xs[t][:], func=AF.Relu,
                            bias=negtau[:, t:t + 1],
                            accum_out=sr[:, t:t + 1])
                        nc.vector.scalar_tensor_tensor(
                            out=sp.tile([128, W], BF16, name=f"sg_{t}", tag="scr")[:],
                            in0=bscr[:], scalar=0.0, in1=bscr[:],
                            op0=OP.add, op1=OP.mult,
                            accum_out=gstat[:, t:t + 1])

            # ---- batched tau2 + S2 assembly ----
            # uniformize ScalarE tiles (cols t%3 in {1,2}): sm = sr + W*tau1
            for r0 in (1, 2):
                cs = slice(r0, NT, 3)
                nc.vector.scalar_tensor_tensor(
                    out=sm[:, cs], in0=tau1[:, cs], scalar=float(W),
                    op0=OP.mult, in1=sr[:, cs], op1=OP.add)
            # sint = sm - W*tau1  (= s - cnt*tau1);  s = sint + cnt*tau1
            nc.vector.scalar_tensor_tensor(out=sint[:], in0=tau1[:],
                                           scalar=-float(W), op0=OP.mult,
                                           in1=sm[:], op1=OP.add)
            nc.vector.tensor_tensor(out=m2t[:], in0=cnt[:], in1=tau1[:],
                                    op=OP.mult)
            nc.vector.tensor_tensor(out=stile[:], in0=sint[:], in1=m2t[:],
                                    op=OP.add)
            nc.vector.tensor_scalar(out=sm1[:], in0=stile[:], scalar1=-1.0,
                                    scalar2=None, op0=OP.add)
            nc.vector.reciprocal(out=rcp[:], in_=cnt[:])
            nc.vector.tensor_tensor(out=tau2[:], in0=sm1[:], in1=rcp[:],
                                    op=OP.mult)
            # S2@tau1: DVE (max) tiles need G -= tau1*sint   (cols 0::3)
            cs = slice(0, NT, 3)
            nc.vector.tensor_tensor(out=cor[:], in0=tau1[:, cs],
                                    in1=sint[:, cs], op=OP.mult)
            nc.vector.tensor_tensor(out=gstat[:, cs], in0=gstat[:, cs],
                                    in1=cor[:], op=OP.subtract)
            # S2@tau2 = S2@tau1 - 2*(tau2-tau1)*s + (tau2^2-tau1^2)*cnt
            nc.vector.tensor_tensor(out=f1t[:], in0=tau2[:], in1=tau1[:],
                                    op=OP.subtract)
            nc.vector.tensor_tensor(out=f2t[:], in0=tau2[:], in1=tau1[:],
                                    op=OP.add)
            nc.vector.tensor_tensor(out=g1t[:], in0=f1t[:], in1=stile[:],
                                    op=OP.mult)
            nc.vector.scalar_tensor_tensor(out=h1t[:], in0=g1t[:], scalar=-2.0,
                                           op0=OP.mult, in1=gstat[:], op1=OP.add)
            nc.vector.tensor_tensor(out=g2t[:], in0=f1t[:], in1=f2t[:],
                                    op=OP.mult)
            nc.vector.tensor_tensor(out=g3t[:], in0=g2t[:], in1=cnt[:],
                                    op=OP.mult)
            nc.vector.tensor_tensor(out=s2[:], in0=h1t[:], in1=g3t[:],
                                    op=OP.add)

            # ---- epilogue: ctr = (sqrt(S2) - S2 - tau2) * valid; reduce ----
            nc.scalar.activation(out=sq[:], in_=s2[:], func=AF.Sqrt)
            nc.vector.tensor_tensor(out=ctr[:], in0=sq[:], in1=s2[:],
                                    op=OP.subtract)
            nc.vector.tensor_tensor(out=ctr2[:], in0=ctr[:], in1=tau2[:],
                                    op=OP.subtract)
            nc.vector.tensor_tensor(out=ctr[:], in0=ctr2[:], in1=val[:],
                                    op=OP.mult)
            nc.vector.tensor_reduce(out=rowtot[:], in_=ctr[:],
                                    axis=mybir.AxisListType.X, op=OP.add)
            tps = tpsum.tile([1, 1], F32, name="tot", tag="tot")
            nc.tensor.matmul(tps[:], lhsT=rowtot[:], rhs=ones128[:],
                             start=True, stop=True)
            nc.vector.tensor_copy(out_sb[:], tps[:])
            nc.sync.dma_start(out_d[:], out_sb[:])

    nc.compile()
    return nc


_NC_CACHE = {}


def _get_nc(W):
    if W not in _NC_CACHE:
        _NC_CACHE[W] = build_graph(W)
    return _NC_CACHE[W]


def window_for(mask):
    max_nu = int(mask.astype(bool).sum(1).max())
    return min(K, ((max_nu + 15) // 16) * 16)


def make_in_maps(g, wq, wk, mask):
    bf16 = ml_dtypes.bfloat16
    W = window_for(mask)
    wqT = np.ascontiguousarray(
        (wq.astype(np.float64) * BETA).transpose(2, 0, 1).reshape(D, H * Z)
    ).astype(bf16)
    wkT = np.ascontiguousarray(
        wk.transpose(2, 0, 1).reshape(D, H * Z)).astype(bf16)
    in_maps = []
    for b in range(B):
        mb = mask[b].astype(bool)
        n_u = int(mb.sum())
        assert n_u <= NQC * 128, "unmasked row count exceeds processed rows"
        perm = np.argsort(~mb, kind="stable")  # unmasked rows first
        gTp = np.ascontiguousarray(g[b].T[:, perm]).astype(bf16)
        maskp = mb[perm]
        vrow = ((maskp.astype(np.float32) - 1.0) * 1000.0)[None, :].astype(bf16)
        base = maskp[:NQC * 128].astype(np.float32).reshape(NQC, 128).T  # [128, NQC]
        val = np.ascontiguousarray(np.tile(base, (1, H)))  # cols t = h*NQC+qc
        params = np.empty((128, 2), dtype=np.float32)
        params[:, 0] = 1000.0 * (W - n_u) - 1.0
        params[:, 1] = 1.0 / n_u
        in_maps.append({"gT": gTp, "wqT": wqT, "wkT": wkT,
                        "vrow": vrow, "val": val, "params": params})
    return in_maps


def combine(partials, mask):
    n_masked_rows = H * (K - mask.sum(1).astype(np.int64))  # per batch
    total = 0.0
    for b in range(B):
        total += float(partials[b]) + MASKED_ROW_E * float(n_masked_rows[b])
    return np.asarray(total / BETA, dtype=np.float32)


def kernel(g, wq, wk, mask):
    mask = np.asarray(mask)
    nc = _get_nc(window_for(mask))
    in_maps = make_in_maps(np.asarray(g, dtype=np.float32),
                           np.asarray(wq, dtype=np.float32),
                           np.asarray(wk, dtype=np.float32),
                           mask)
    res = run_bass_kernel_spmd(nc, in_maps, core_ids=list(range(8)))
    partials = [np.asarray(res.results[b]["out"], dtype=np.float64).reshape(-1)[0]
                for b in range(B)]
    return combine(partials, mask)


# revision 17
# speedup vs baseline: 1.0374x; 1.0152x over previous
"""Trainium2 Bass kernel for nn_Attention_75849122447825 (sparse_attention).

Math: reference computes, per (b,h) head, scores x = beta * (q g)(k g)^T with a
pair mask, sparsemax over the last axis, and the scalar energy
    e = -sum_rows( <x,p> - ||p||_2 ),  output = e / beta.

Key identities (p = sparsemax(x) row-wise, sum_k p = 1):
  <x,p> = ||p||^2 + tau            (x = p + tau on the support)
  row contribution to e:  sqrt(S2) - S2 - tau,  S2 = sum relu(x - tau)^2
Masked query rows (mask[q]=0) are constant rows x = -125000; the reference's
f32 arithmetic gives them the exact constant contribution
  C = 500000 + sqrt(0.03125)   (tau_f32 = -125000.0078125, p = 2^-7 uniform,
                                support 134  ->  <x,p> = -500000, ||p|| = 2^-2.5)
so only unmasked rows need device compute; masked rows are counted on host.

tau per row via Michelot's algorithm (tau' = (sum_{x>tau} x - 1)/#{x>tau}).
The first iterate is analytic: at any tau in (-1000, min_real_x) the support is
exactly the n_u real columns, so tau1 = (rowsum + 1000*(W-n_u) - 1)/n_u comes
free from the rowsum fused into the PSUM->SBUF copy. One paid stats pass at
tau1 gives, per A tile (fused accum ops):
  cnt = #{x > tau1}                               (DVE is_gt)
  B:   m = max(x,tau1), sm = sum m   [DVE tiles]  or
       r = relu(x-tau1), sr = sum r  [ScalarE tiles]
  G:   sum (m-tau1)*m  (= S2@tau1 + tau1*(s-c*tau1))   [reads B's scratch]
       or sum r*r      (= S2@tau1)
Then batch math: s = sm - (W-c)*tau1, tau2 = (s-1)/cnt, and
S2@tau2 = S2@tau1 - 2*(tau2-tau1)*s + (tau2^2-tau1^2)*cnt (support sets are
stable at convergence) — verified to reproduce the f32 reference exactly.

Sharding: data-parallel over batch B=8, one batch per NeuronCore; host combines
the 8 partial sums plus the analytic masked-row term. Host pre-permutes each
batch's rows so unmasked rows come first and pre-transposes g. Masked key
columns get a -1000 fill through 65-row augmented projection tiles (qp row 64
= ones, kp row 64 = v). All real columns land in the leading max_b(n_u)
positions, so every elementwise/stats pass runs on a trimmed column window W
(=272 here; the graph is built for the W derived from the actual mask, cached).
The trace is emitted per 2-head group (projection -> A tiles -> tau1 -> stats)
so the engines' in-order streams pipeline across groups instead of phase
barriers.
"""

import math
import numpy as np
import ml_dtypes

import concourse.bass as bass
import concourse.tile as tile
from concourse import bacc, mybir
from concourse.bass_utils import run_bass_kernel_spmd

# problem constants (hardcoded per task rules)
B, K, D, H, Z = 8, 512, 768, 12, 64
BETA = 1.0 / math.sqrt(Z)
DC = D // 128          # 6 d-chunks
MG = (H * Z) // 128    # 6 m-groups (2 heads each)
NQC = 3                # q-row chunks of 128 processed (384 rows >= n_u always here)
NT = H * NQC           # 36 A-tiles
MASKED_ROW_E = 500000.0 + math.sqrt(0.03125)  # exact f32 reference behavior
NITERS = 1  # informational: one paid stats pass after the analytic tau1

BF16 = mybir.dt.bfloat16
F32 = mybir.dt.float32
OP = mybir.AluOpType
AF = mybir.ActivationFunctionType


def build_graph(W):
    assert W % 16 == 0 and 0 < W <= K
    nc = bacc.Bacc("TRN2", target_bir_lowering=False, debug=False,
                   enable_asserts=False, num_devices=8)

    gT_d = nc.dram_tensor("gT", [D, K], BF16, kind="ExternalInput")
    wqT_d = nc.dram_tensor("wqT", [D, H * Z], BF16, kind="ExternalInput")
    wkT_d = nc.dram_tensor("wkT", [D, H * Z], BF16, kind="ExternalInput")
    vrow_d = nc.dram_tensor("vrow", [1, K], BF16, kind="ExternalInput")
    val_d = nc.dram_tensor("val", [128, NT], F32, kind="ExternalInput")
    # params: col0 = 1000*(W-n_u)-1, col1 = 1/n_u   (replicated down partitions)
    params_d = nc.dram_tensor("params", [128, 2], F32, kind="ExternalInput")
    out_d = nc.dram_tensor("out", [1, 1], F32, kind="ExternalOutput")

    with tile.TileContext(nc) as tc:
        with (
            tc.tile_pool(name="persist", bufs=1) as pp,
            tc.tile_pool(name="scr", bufs=8) as sp,
            tc.tile_pool(name="psum", bufs=2, space="PSUM") as qpsum,
            tc.tile_pool(name="apsum", bufs=5, space="PSUM") as apsum,
            tc.tile_pool(name="tpsum", bufs=1, space="PSUM") as tpsum,
        ):
            # ---- persistent SBUF tiles ----
            gT = [pp.tile([128, K], BF16, name=f"gT{i}", tag=f"gT{i}")
                  for i in range(DC)]
            wqT = [pp.tile([128, H * Z], BF16, name=f"wqT{i}", tag=f"wqT{i}")
                   for i in range(DC)]
            wkT = [pp.tile([128, H * Z], BF16, name=f"wkT{i}", tag=f"wkT{i}")
                   for i in range(DC)]
            # 65-row augmented projections: qp row 64 = ones, kp row 64 = v
            QCOLS = NQC * 128
            qp = [pp.tile([65, QCOLS], BF16, name=f"qp{h}", tag=f"qp{h}")
                  for h in range(H)]
            kp = [pp.tile([65, W], BF16, name=f"kp{h}", tag=f"kp{h}")
                  for h in range(H)]
            xs = [pp.tile([128, W], BF16, name=f"x{t}", tag=f"x{t}")
                  for t in range(NT)]
            val = pp.tile([128, NT], F32, name="val", tag="val")
            params = pp.tile([128, 2], F32, name="params", tag="params")
            rowsum = pp.tile([128, NT], F32, name="rowsum", tag="rowsum")
            rs1 = pp.tile([128, NT], F32, name="rs1", tag="rs1")
            cnt = pp.tile([128, NT], F32, name="cnt", tag="cnt")
            sm = pp.tile([128, NT], F32, name="sm", tag="sm")
            sr = pp.tile([128, NT], F32, name="sr", tag="sr")
            gstat = pp.tile([128, NT], F32, name="gstat", tag="gstat")
            tau1 = pp.tile([128, NT], F32, name="tau1", tag="tau1")
            tau2 = pp.tile([128, NT], F32, name="tau2", tag="tau2")
            negtau = pp.tile([128, NT], F32, name="negtau", tag="negtau")
            sint = pp.tile([128, NT], F32, name="sint", tag="sint")
            stile = pp.tile([128, NT], F32, name="stile", tag="stile")
            sm1 = pp.tile([128, NT], F32, name="sm1", tag="sm1")
            rcp = pp.tile([128, NT], F32, name="rcp", tag="rcp")
            m2t = pp.tile([128, NT], F32, name="m2t", tag="m2t")
            cor = pp.tile([128, 12], F32, name="cor", tag="cor")
            f1t = pp.tile([128, NT], F32, name="f1t", tag="f1t")
            f2t = pp.tile([128, NT], F32, name="f2t", tag="f2t")
            g1t = pp.tile([128, NT], F32, name="g1t", tag="g1t")
            h1t = pp.tile([128, NT], F32, name="h1t", tag="h1t")
            g2t = pp.tile([128, NT], F32, name="g2t", tag="g2t")
            g3t = pp.tile([128, NT], F32, name="g3t", tag="g3t")
            s2 = pp.tile([128, NT], F32, name="s2", tag="s2")
            sq = pp.tile([128, NT], F32, name="sq", tag="sq")
            ctr = pp.tile([128, NT], F32, name="ctr", tag="ctr")
            ctr2 = pp.tile([128, NT], F32, name="ctr2", tag="ctr2")
            rowtot = pp.tile([128, 1], F32, name="rowtot", tag="rowtot")
            ones128 = pp.tile([128, 1], F32, name="ones128", tag="ones128")
            out_sb = pp.tile([1, 1], F32, name="out_sb", tag="out_sb")

            # ---- input DMAs + constants ----
            for i in range(DC):
                nc.sync.dma_start(gT[i][:], gT_d[i * 128:(i + 1) * 128, :])
                nc.sync.dma_start(wqT[i][:], wqT_d[i * 128:(i + 1) * 128, :])
                nc.sync.dma_start(wkT[i][:], wkT_d[i * 128:(i + 1) * 128, :])
            nc.sync.dma_start(val[:], val_d[:])
            nc.sync.dma_start(params[:], params_d[:])
            nc.vector.memset(ones128[:], 1.0)
            for h in range(H):
                nc.gpsimd.memset(qp[h][64:65, 0:QCOLS], 1.0)
                nc.sync.dma_start(kp[h][64:65, 0:W], vrow_d[0:1, 0:W])

            # ---- pipelined main loop: per 2-head group ----
            # proj(mg+1) is emitted before stats(mg) so ACT's proj copies are
            # not stuck behind the previous group's relu passes
            def emit_proj(mg):
                for w_sb, p_sb, ncols in ((wqT, qp, QCOLS), (wkT, kp, W)):
                    ps = qpsum.tile([128, ncols], F32,
                                    name=f"proj{mg}_{ncols}", tag="proj")
                    for dc in range(DC):
                        nc.tensor.matmul(
                            ps[:],
                            lhsT=w_sb[dc][:, mg * 128:(mg + 1) * 128],
                            rhs=gT[dc][:, 0:ncols],
                            start=(dc == 0), stop=(dc == DC - 1),
                        )
                    nc.scalar.copy(p_sb[2 * mg][0:64, :], ps[0:64, :])
                    nc.scalar.copy(p_sb[2 * mg + 1][0:64, :], ps[64:128, :])

            emit_proj(0)
            for mg in range(MG):
                g0 = 6 * mg
                for h in (2 * mg, 2 * mg + 1):
                    for qc in range(NQC):
                        t = h * NQC + qc
                        aps = apsum.tile([128, W], F32, name=f"a{t}", tag="a")
                        nc.tensor.matmul(
                            aps[:], lhsT=qp[h][:, qc * 128:(qc + 1) * 128],
                            rhs=kp[h][:], start=True, stop=True)
                        if t % 3 == 0:
                            nc.vector.tensor_scalar(
                                out=xs[t][:], in0=aps[:], scalar1=0.0,
                                scalar2=None, op0=OP.add, op1=OP.add,
                                accum_out=rowsum[:, t:t + 1])
                        else:
                            nc.scalar.activation(
                                out=xs[t][:], in_=aps[:], func=AF.Identity,
                                accum_out=rowsum[:, t:t + 1])

                if mg + 1 < MG:
                    emit_proj(mg + 1)

                # group tau1 = (rowsum + 1000*(W-n_u) - 1) / n_u ; negtau
                gs = slice(g0, g0 + 6)
                nc.vector.tensor_scalar(out=rs1[:, gs], in0=rowsum[:, gs],
                                        scalar1=params[:, 0:1], scalar2=None,
                                        op0=OP.add)
                nc.vector.tensor_scalar(out=tau1[:, gs], in0=rs1[:, gs],
                                        scalar1=params[:, 1:2], scalar2=None,
                                        op0=OP.mult)
                nc.vector.tensor_scalar(out=negtau[:, gs], in0=tau1[:, gs],
                                        scalar1=-1.0, scalar2=None, op0=OP.mult)

                # stats passes at tau1
                for t in range(g0, g0 + 6):
                    nc.vector.tensor_scalar(
                        out=sp.tile([128, W], BF16, name=f"sc_{t}", tag="scr")[:],
                        in0=xs[t][:], scalar1=tau1[:, t:t + 1], scalar2=None,
                        op0=OP.is_gt, op1=OP.add, accum_out=cnt[:, t:t + 1])
                    bscr = sp.tile([128, W], BF16, name=f"sb_{t}", tag="scr")
                    if t % 3 == 0:
                        nc.vector.tensor_scalar(
                            out=bscr[:], in0=xs[t][:],
                            scalar1=tau1[:, t:t + 1], scalar2=None,
                            op0=OP.max, op1=OP.add, accum_out=sm[:, t:t + 1])
                        nc.vector.scalar_tensor_tensor(
                            out=sp.tile([128, W], BF16, name=f"sg_{t}", tag="scr")[:],
                            in0=bscr[:], scalar=tau1[:, t:t + 1], in1=bscr[:],
                            op0=OP.subtract, op1=OP.mult,
                            accum_out=gstat[:, t:t + 1])
                    else:
                        nc.scalar.activation(
                            out=bscr[:], in_=xs[t][:], func=AF.Relu,
                            bias=negtau[:, t:t + 1],
                            accum_out=sr[:, t:t + 1])
                        nc.vector.scalar_tensor_tensor(
                            out=sp.tile([128, W], BF16, name=f"sg_{t}", tag="scr")[:],
                            in0=bscr[:], scalar=0.0, in1=bscr[:],
                            op0=OP.add, op1=OP.mult,
                            accum_out=gstat[:, t:t + 1])

            # ---- batched tau2 + S2 assembly ----
            # uniformize ScalarE tiles (cols t%3 in {1,2}): sm = sr + W*tau1
            for r0 in (1, 2):
                cs = slice(r0, NT, 3)
                nc.vector.scalar_tensor_tensor(
                    out=sm[:, cs], in0=tau1[:, cs], scalar=float(W),
                    op0=OP.mult, in1=sr[:, cs], op1=OP.add)
            # sint = sm - W*tau1  (= s - cnt*tau1);  s = sint + cnt*tau1
            nc.vector.scalar_tensor_tensor(out=sint[:], in0=tau1[:],
                                           scalar=-float(W), op0=OP.mult,
                                           in1=sm[:], op1=OP.add)
            nc.vector.tensor_tensor(out=m2t[:], in0=cnt[:], in1=tau1[:],
                                    op=OP.mult)
            nc.vector.tensor_tensor(out=stile[:], in0=sint[:], in1=m2t[:],
                                    op=OP.add)
            nc.vector.tensor_scalar(out=sm1[:], in0=stile[:], scalar1=-1.0,
                                    scalar2=None, op0=OP.add)
            nc.vector.reciprocal(out=rcp[:], in_=cnt[:])
            nc.vector.tensor_tensor(out=tau2[:], in0=sm1[:], in1=rcp[:],
                                    op=OP.mult)
            # S2@tau1: DVE (max) tiles need G -= tau1*sint   (cols 0::3)
            cs = slice(0, NT, 3)
            nc.vector.tensor_tensor(out=cor[:], in0=tau1[:, cs],
                                    in1=sint[:, cs], op=OP.mult)
            nc.vector.tensor_tensor(out=gstat[:, cs], in0=gstat[:, cs],
                                    in1=cor[:], op=OP.subtract)
            # S2@tau2 = S2@tau1 - 2*(tau2-tau1)*s + (tau2^2-tau1^2)*cnt
            nc.vector.tensor_tensor(out=f1t[:], in0=tau2[:], in1=tau1[:],
                                    op=OP.subtract)
            nc.vector.tensor_tensor(out=f2t[:], in0=tau2[:], in1=tau1[:],
                                    op=OP.add)
            nc.vector.tensor_tensor(out=g1t[:], in0=f1t[:], in1=stile[:],
                                    op=OP.mult)
            nc.vector.scalar_tensor_tensor(out=h1t[:], in0=g1t[:], scalar=-2.0,
                                           op0=OP.mult, in1=gstat[:], op1=OP.add)
            nc.vector.tensor_tensor(out=g2t[:], in0=f1t[:], in1=f2t[:],
                                    op=OP.mult)
            nc.vector.tensor_tensor(out=g3t[:], in0=g2t[:], in1=cnt[:],
                                    op=OP.mult)
            nc.vector.tensor_tensor(out=s2[:], in0=h1t[:], in1=g3t[:],
                                    op=OP.add)

            # ---- epilogue: ctr = (sqrt(S2) - S2 - tau2) * valid; reduce ----
            nc.scalar.activation(out=sq[:], in_=s2[:], func=AF.Sqrt)
            nc.vector.tensor_tensor(out=ctr[:], in0=sq[:], in1=s2[:],
                                    op=OP.subtract)
            nc.vector.tensor_tensor(out=ctr2[:], in0=ctr[:], in1=tau2[:],
                                    op=OP.subtract)
            nc.vector.tensor_tensor(out=ctr[:], in0=ctr2[:], in1=val[:],
                                    op=OP.mult)
            nc.vector.tensor_reduce(out=rowtot[:], in_=ctr[:],
                                    axis=mybir.AxisListType.X, op=OP.add)
            tps = tpsum.tile([1, 1], F32, name="tot", tag="tot")
            nc.tensor.matmul(tps[:], lhsT=rowtot[:], rhs=ones128[:],
                             start=True, stop=True)
            nc.vector.tensor_copy(out_sb[:], tps[:])
            nc.sync.dma_start(out_d[:], out_sb[:])

    nc.compile()
    return nc


_NC_CACHE = {}


def _get_nc(W):
    if W not in _NC_CACHE:
        _NC_CACHE[W] = build_graph(W)
    return _NC_CACHE[W]


def window_for(mask):
    max_nu = int(mask.astype(bool).sum(1).max())
    return min(K, ((max_nu + 15) // 16) * 16)


def make_in_maps(g, wq, wk, mask):
    bf16 = ml_dtypes.bfloat16
    W = window_for(mask)
    wqT = np.ascontiguousarray(
        (wq.astype(np.float64) * BETA).transpose(2, 0, 1).reshape(D, H * Z)
    ).astype(bf16)
    wkT = np.ascontiguousarray(
        wk.transpose(2, 0, 1).reshape(D, H * Z)).astype(bf16)
    in_maps = []
    for b in range(B):
        mb = mask[b].astype(bool)
        n_u = int(mb.sum())
        assert n_u <= NQC * 128, "unmasked row count exceeds processed rows"
        perm = np.argsort(~mb, kind="stable")  # unmasked rows first
        gTp = np.ascontiguousarray(g[b].T[:, perm]).astype(bf16)
        maskp = mb[perm]
        vrow = ((maskp.astype(np.float32) - 1.0) * 1000.0)[None, :].astype(bf16)
        base = maskp[:NQC * 128].astype(np.float32).reshape(NQC, 128).T  # [128, NQC]
        val = np.ascontiguousarray(np.tile(base, (1, H)))  # cols t = h*NQC+qc
        params = np.empty((128, 2), dtype=np.float32)
        params[:, 0] = 1000.0 * (W - n_u) - 1.0
        params[:, 1] = 1.0 / n_u
        in_maps.append({"gT": gTp, "wqT": wqT, "wkT": wkT,
                        "vrow": vrow, "val": val, "params": params})
    return in_maps


def combine(partials, mask):
    n_masked_rows = H * (K - mask.sum(1).astype(np.int64))  # per batch
    total = 0.0
    for b in range(B):
        total += float(partials[b]) + MASKED_ROW_E * float(n_masked_rows[b])
    return np.asarray(total / BETA, dtype=np.float32)


def kernel(g, wq, wk, mask):
    mask = np.asarray(mask)
    nc = _get_nc(window_for(mask))
    in_maps = make_in_maps(np.asarray(g, dtype=np.float32),
                           np.asarray(wq, dtype=np.float32),
                           np.asarray(wk, dtype=np.float32),
                           mask)
    res = run_bass_kernel_spmd(nc, in_maps, core_ids=list(range(8)))
    partials = [np.asarray(res.results[b]["out"], dtype=np.float64).reshape(-1)[0]
                for b in range(B)]
    return combine(partials, mask)
